# revision 36
# baseline (speedup 1.0000x reference)
"""Multi-head attention (B=2,S=2048,D=1024,H=16) on 8 TRN2 NeuronCores.

Sharding: core c handles batch b=c//4 and head-group g=c%4 (4 heads,
projection features [256g, 256g+256)).  Each core computes its QKV
projections, attention, and a rank-256 partial of the fc output; the host
sums the 4 partials per batch and adds fc_b.

Layouts on device (per core):
  qhT/khT : feature-major [depth, seq], 2 heads packed per 128-partition tile
  vh      : seq-major [128 seq, head, 65] with a ones column (fused denom)
  scoresT : [k, q]; head pair computed CONCURRENTLY in the PE array via
            row tile_position (K=64 each) into one 2-bank PSUM tile
  exp     : single ACT op per [128, 1024] PSUM tile (scale=1/8 fused)
  attnV   : out[65, 512] = [vh | 1]^T @ p  (row 64 = softmax denominator)
  concatT : normalized; stored f32 (feeds exact attn_products transposes)
            and fp16 (feeds the fc matmul).
Matmul compute dtype is fp16 (PSUM accumulation fp32).
"""
import numpy as np
import ml_dtypes

import concourse.bass as bass
import concourse.tile as tile
from concourse import bacc, bass_utils, mybir
from concourse.bass import ts
from concourse.masks import make_identity

B, S, D, H = 2, 2048, 1024, 16
DEPTH = 64
NCORES = 8
HPC = 4               # heads per core
FS = HPC * DEPTH      # 256 features per core
QB = 512              # q block (matmul moving dim)
NQB = S // QB         # 4
NKT = S // 128        # 16 k tiles
IC = D // 128         # 8 input chunks
SCALE = 1.0 / np.sqrt(DEPTH)

F32 = mybir.dt.float32
BF16 = mybir.dt.float16
NPBF16 = np.float16

RUN_KWARGS = {}       # test.py may inject trace=True etc.
_CACHE = {}


def build():
    nc = bacc.Bacc("TRN2", target_bir_lowering=False)

    xq = nc.dram_tensor("xq", [IC, NQB // 2, 128, 2 * QB], BF16, kind="ExternalInput")
    xk = nc.dram_tensor("xk", [IC, NQB // 2, 128, 2 * QB], BF16, kind="ExternalInput")
    xv = nc.dram_tensor("xv", [IC, NQB // 2, 128, 2 * QB], BF16, kind="ExternalInput")
    wqT = nc.dram_tensor("wqT", [D, FS], BF16, kind="ExternalInput")
    wkT = nc.dram_tensor("wkT", [D, FS], BF16, kind="ExternalInput")
    wvT = nc.dram_tensor("wvT", [D, FS], BF16, kind="ExternalInput")
    fcT = nc.dram_tensor("fcT", [FS, D], BF16, kind="ExternalInput")
    qbv = nc.dram_tensor("qbv", [128, 2], F32, kind="ExternalInput")
    kbv = nc.dram_tensor("kbv", [128, 2], F32, kind="ExternalInput")
    vbv = nc.dram_tensor("vbv", [FS], F32, kind="ExternalInput")
    fco = nc.dram_tensor("fco", [NKT, 128, 2, QB], F32, kind="ExternalOutput")
    atp = nc.dram_tensor("atp", [NKT // 2, 128, 2, HPC, DEPTH], F32, kind="ExternalOutput")

    with tile.TileContext(nc) as tc:
        with tc.tile_pool(name="persist", bufs=1) as pp:
            wq_sb = pp.tile([128, IC, FS], BF16, name="wq_sb")
            wk_sb = pp.tile([128, IC, FS], BF16, name="wk_sb")
            wv_sb = pp.tile([128, IC, FS], BF16, name="wv_sb")
            fc_sb = pp.tile([128, 2, D], BF16, name="fc_sb")
            qbias = pp.tile([128, 2], F32, name="qbias")
            kbias = pp.tile([128, 2], F32, name="kbias")
            vbias_bc = pp.tile([128, FS], F32, name="vbias_bc")
            ident = pp.tile([128, 64], F32, name="ident")
            qh = [pp.tile([128, S], BF16, name=f"qh{t}") for t in range(2)]
            kh = [pp.tile([128, S], BF16, name=f"kh{t}") for t in range(2)]
            cc32 = [pp.tile([128, S], F32, name=f"cc32_{t}") for t in range(2)]
            cc16 = [pp.tile([128, S], BF16, name=f"cc16_{t}") for t in range(2)]
            vh_sb = pp.tile([128, NKT, HPC, DEPTH + 1], BF16, name="vh_sb")

            for i in range(IC):
                nc.sync.dma_start(out=wk_sb[:, i, :], in_=wkT[ts(i, 128), :])
            nc.sync.dma_start(out=kbias[:, :], in_=kbv[:, :])
            nc.gpsimd.dma_start(out=vbias_bc[:, :], in_=vbv[None, :].to_broadcast([128, FS]))
            make_identity(nc, ident[0:64, :])
            nc.vector.tensor_copy(ident[64:128, :], ident[0:64, :])
            nc.vector.memset(vh_sb[:, :, :, DEPTH:DEPTH + 1], 1.0)

            # ---- Phase A: projections ----
            with (
                tc.tile_pool(name="xin", bufs=3) as xp,
                tc.tile_pool(name="psA", bufs=2, space="PSUM") as psA,
                tc.tile_pool(name="psV", bufs=2, space="PSUM") as psV,
            ):
                for sbp in range(NQB // 2):
                    x_t = xp.tile([128, IC, 2 * QB], BF16, name="x_t", tag="xin")
                    for i in range(IC):
                        nc.sync.dma_start(out=x_t[:, i, :], in_=xk[i, sbp, :, :])
                    for half in range(2):
                        sb_i = 2 * sbp + half
                        for t in range(2):
                            ps = psA.tile([128, QB], F32, name="ps_t", tag="psA")
                            for i in range(IC):
                                nc.tensor.matmul(ps[:, :], wk_sb[:, i, ts(t, 128)],
                                                 x_t[:, i, ts(half, QB)],
                                                 start=(i == 0), stop=(i == IC - 1))
                            nc.vector.tensor_scalar_add(kh[t][:, ts(sb_i, QB)], ps[:, :], kbias[:, t:t + 1])
                for i in range(IC):
                    nc.sync.dma_start(out=wv_sb[:, i, :], in_=wvT[ts(i, 128), :])
                for sbp in range(NQB // 2):
                    x_t = xp.tile([128, IC, 2 * QB], BF16, name="x_t", tag="xin")
                    for i in range(IC):
                        nc.sync.dma_start(out=x_t[:, i, :], in_=xv[i, sbp, :, :])
                    for st in range(2 * QB // 128):
                        psv = psV.tile([128, FS], F32, name="psv_t", tag="psV")
                        for i in range(IC):
                            nc.tensor.matmul(psv[:, :], x_t[:, i, ts(st, 128)], wv_sb[:, i, :],
                                             start=(i == 0), stop=(i == IC - 1))
                        kt = sbp * (2 * QB // 128) + st
                        nc.vector.tensor_add(
                            vh_sb[:, kt, :, 0:DEPTH],
                            psv.rearrange("p (h d) -> p h d", h=HPC),
                            vbias_bc.rearrange("p (h d) -> p h d", h=HPC),
                        )
                for i in range(IC):
                    nc.sync.dma_start(out=wq_sb[:, i, :], in_=wqT[ts(i, 128), :])
                nc.sync.dma_start(out=qbias[:, :], in_=qbv[:, :])
                for i in range(2):
                    nc.sync.dma_start(out=fc_sb[:, i, :], in_=fcT[ts(i, 128), :])
                for sbp in range(NQB // 2):
                    x_t = xp.tile([128, IC, 2 * QB], BF16, name="x_t", tag="xin")
                    for i in range(IC):
                        nc.sync.dma_start(out=x_t[:, i, :], in_=xq[i, sbp, :, :])
                    for half in range(2):
                        sb_i = 2 * sbp + half
                        for t in range(2):
                            ps = psA.tile([128, QB], F32, name="ps_t", tag="psA")
                            for i in range(IC):
                                nc.tensor.matmul(ps[:, :], wq_sb[:, i, ts(t, 128)],
                                                 x_t[:, i, ts(half, QB)],
                                                 start=(i == 0), stop=(i == IC - 1))
                            nc.vector.tensor_scalar_add(qh[t][:, ts(sb_i, QB)], ps[:, :], qbias[:, t:t + 1])

            # ---- Phase B: attention, head pairs concurrent in the PE array ----
            with (
                tc.tile_pool(name="psS", bufs=2, space="PSUM") as psS,
                tc.tile_pool(name="psAV", bufs=4, space="PSUM") as psAV,
                tc.tile_pool(name="pP", bufs=4) as pP,
                tc.tile_pool(name="pR", bufs=4) as pR,
            ):
                for qb_i in range(NQB):
                    for hp in range(2):
                        h0, h1 = 2 * hp, 2 * hp + 1
                        av0 = psAV.tile([DEPTH + 1, QB], F32, name="av0_t", tag="av")
                        av1 = psAV.tile([DEPTH + 1, QB], F32, name="av1_t", tag="av")
                        for kt in range(NKT):
                            sp = psS.tile([128, 2 * QB], F32, name="sp_t", tag="s")
                            nc.tensor.matmul(sp[:, 0:QB], kh[hp][0:64, ts(kt, 128)],
                                             qh[hp][0:64, ts(qb_i, QB)],
                                             start=True, stop=True)
                            nc.tensor.matmul(sp[:, QB:2 * QB], kh[hp][64:128, ts(kt, 128)],
                                             qh[hp][64:128, ts(qb_i, QB)],
                                             start=True, stop=True)
                            p_t = pP.tile([128, 2 * QB], BF16, name="p_t", tag="p")
                            nc.scalar.activation(out=p_t[:, :], in_=sp[:, :],
                                                 func=mybir.ActivationFunctionType.Exp,
                                                 scale=float(SCALE))
                            nc.tensor.matmul(av0[:, :], vh_sb[:, kt, h0, :], p_t[:, 0:QB],
                                             start=(kt == 0), stop=(kt == NKT - 1))
                            nc.tensor.matmul(av1[:, :], vh_sb[:, kt, h1, :], p_t[:, QB:2 * QB],
                                             start=(kt == 0), stop=(kt == NKT - 1))
                        for h, av in ((h0, av0), (h1, av1)):
                            po = 64 * (h % 2)
                            d0 = pR.tile([1, QB], F32, name="d0_t", tag="d0")
                            nc.vector.tensor_copy(d0[0:1, :], av[DEPTH:DEPTH + 1, :])
                            rb = pR.tile([64, QB], F32, name="rb_t", tag="rb")
                            nc.gpsimd.partition_broadcast(rb[:, :], d0[0:1, :])
                            rr = pR.tile([64, QB], F32, name="rr_t", tag="rr")
                            nc.vector.reciprocal_approx_fast(out=rr[:, :], in_=rb[:, :])
                            nc.vector.tensor_mul(cc32[hp][po:po + 64, ts(qb_i, QB)],
                                                 av[0:DEPTH, :], rr[:, :])
                        nc.vector.tensor_copy(cc16[hp][:, ts(qb_i, QB)],
                                              cc32[hp][:, ts(qb_i, QB)])

            # ---- Phase C: fc partial, Phase D: attn_products out ----
            with (
                tc.tile_pool(name="psF", bufs=3, space="PSUM") as psF,
                tc.tile_pool(name="psT", bufs=3, space="PSUM") as psT,
                tc.tile_pool(name="pO", bufs=3) as pO,
                tc.tile_pool(name="pA", bufs=3) as pA,
            ):
                for stp in range(NKT // 2):
                    at_t = pA.tile([128, 2, HPC, DEPTH], F32, name="at_t", tag="at")
                    for j in range(2):
                        st = 2 * stp + j
                        o2 = pO.tile([128, 2, QB], F32, name="o2", tag="o")
                        for oh in range(2):
                            fps = psF.tile([128, QB], F32, name="fps", tag="f")
                            for icc in range(2):
                                nc.tensor.matmul(fps[:, :], cc16[icc][:, ts(st, 128)],
                                                 fc_sb[:, icc, ts(oh, QB)],
                                                 start=(icc == 0), stop=(icc == 1))
                            nc.scalar.copy(o2[:, oh, :], fps[:, :])
                        nc.sync.dma_start(out=fco[st, :, :, :], in_=o2[:, :, :])
                        for h in range(HPC):
                            t, po = h // 2, 64 * (h % 2)
                            tp = psT.tile([128, 64], F32, name="tp", tag="t")
                            nc.tensor.matmul(tp[:, :], cc32[t][po:po + 64, ts(st, 128)],
                                             ident[po:po + 64, :], is_transpose=True)
                            nc.vector.tensor_copy(at_t[:, j, h, :], tp[:, :])
                    nc.sync.dma_start(out=atp[stp, :, :, :, :], in_=at_t[:, :, :, :])

    nc.finalize()
    return nc


def _get_nc():
    if "nc" not in _CACHE:
        _CACHE["nc"] = build()
    return _CACHE["nc"]


def _numpy_reference(q, k, v, mask, wq_w, wq_b, wk_w, wk_b, wv_w, wv_b, fc_w, fc_b):
    """Exact fallback for the (never expected) nonzero-mask case."""
    def split_heads(x):
        b, s, _ = x.shape
        return x.reshape(b, s, H, DEPTH).transpose(0, 2, 1, 3)

    qh = split_heads(q @ wq_w.T + wq_b)
    kh = split_heads(k @ wk_w.T + wk_b)
    vh = split_heads(v @ wv_w.T + wv_b)
    scores = np.einsum("bhqd,bhkd->bhqk", qh, kh) / np.sqrt(DEPTH)
    scores = scores + mask * (-1e9)
    scores -= scores.max(axis=-1, keepdims=True)
    e = np.exp(scores)
    attn = e / e.sum(axis=-1, keepdims=True)
    ap = np.einsum("bhqk,bhkd->bhqd", attn, vh).transpose(0, 2, 1, 3)
    concat = ap.reshape(q.shape[0], q.shape[1], D)
    return (concat @ fc_w.T + fc_b).astype(np.float32), ap.astype(np.float32)


def kernel(q, k, v, mask, wq_w, wq_b, wk_w, wk_b, wv_w, wv_b, fc_w, fc_b):
    q = np.asarray(q, np.float32)
    k = np.asarray(k, np.float32)
    v = np.asarray(v, np.float32)
    mask = np.asarray(mask, np.float32)
    wq_w = np.asarray(wq_w, np.float32)
    wq_b = np.asarray(wq_b, np.float32)
    wk_w = np.asarray(wk_w, np.float32)
    wk_b = np.asarray(wk_b, np.float32)
    wv_w = np.asarray(wv_w, np.float32)
    wv_b = np.asarray(wv_b, np.float32)
    fc_w = np.asarray(fc_w, np.float32)
    fc_b = np.asarray(fc_b, np.float32)

    if mask.any():
        return _numpy_reference(q, k, v, mask, wq_w, wq_b, wk_w, wk_b,
                                wv_w, wv_b, fc_w, fc_b)

    def blk(x):
        # [S, D] -> feature-major paired blocks [IC, NQB//2, 128, 2*QB]
        xt = x.T.astype(NPBF16)
        return np.ascontiguousarray(
            xt.reshape(IC, 128, NQB // 2, 2 * QB).transpose(0, 2, 1, 3))

    qT = [blk(q[b]) for b in range(B)]
    kT = [blk(k[b]) for b in range(B)]
    vT = [blk(v[b]) for b in range(B)]
    per_g = []
    for g in range(NCORES // B):
        sl = slice(FS * g, FS * g + FS)
        per_g.append({
            "wqT": np.ascontiguousarray(wq_w[sl, :].T).astype(NPBF16),
            "wkT": np.ascontiguousarray(wk_w[sl, :].T).astype(NPBF16),
            "wvT": np.ascontiguousarray(wv_w[sl, :].T).astype(NPBF16),
            "fcT": np.ascontiguousarray(fc_w[:, sl].T).astype(NPBF16),
            "qbv": np.ascontiguousarray(wq_b[sl].reshape(2, 128).T),
            "kbv": np.ascontiguousarray(wk_b[sl].reshape(2, 128).T),
            "vbv": np.ascontiguousarray(wv_b[sl]),
        })
    in_maps = []
    for c in range(NCORES):
        b, g = c // 4, c % 4
        m = {"xq": qT[b], "xk": kT[b], "xv": vT[b]}
        m.update(per_g[g])
        in_maps.append(m)

    res = bass_utils.run_bass_kernel_spmd(_get_nc(), in_maps,
                                          core_ids=list(range(NCORES)),
                                          **RUN_KWARGS)
    _CACHE["last_result"] = res

    outputs = np.zeros((B, S, D), np.float32)
    attn = np.empty((B, S, H, DEPTH), np.float32)
    for c in range(NCORES):
        b, g = c // 4, c % 4
        outputs[b] += res.results[c]["fco"].reshape(S, D)
        attn[b, :, HPC * g:HPC * g + HPC, :] = res.results[c]["atp"].transpose(0, 2, 1, 3, 4).reshape(S, HPC, DEPTH)
    outputs += fc_b
    return outputs, attn


# revision 38
# speedup vs baseline: 1.0397x; 1.0397x over previous
"""Multi-head attention (B=2,S=2048,D=1024,H=16) on 8 TRN2 NeuronCores.

Sharding: core c handles batch b=c//4 and head-group g=c%4 (4 heads,
projection features [256g, 256g+256)).  Each core computes its QKV
projections, attention, and a rank-256 partial of the fc output; the host
sums the 4 partials per batch and adds fc_b.

Layouts on device (per core):
  qhT/khT : feature-major [depth, seq], 2 heads packed per 128-partition tile
  vh      : seq-major [128 seq, head, 65] with a ones column (fused denom)
  scoresT : [k, q]; head pair computed CONCURRENTLY in the PE array via
            row tile_position (K=64 each) into one 2-bank PSUM tile
  exp     : single ACT op per [128, 1024] PSUM tile (scale=1/8 fused)
  attnV   : out[65, 512] = [vh | 1]^T @ p  (row 64 = softmax denominator)
  concatT : normalized; stored f32 (feeds exact attn_products transposes)
            and fp16 (feeds the fc matmul).
Matmul compute dtype is fp16 (PSUM accumulation fp32).
"""
import numpy as np
import ml_dtypes

import concourse.bass as bass
import concourse.tile as tile
from concourse import bacc, bass_utils, mybir
from concourse.bass import ts
from concourse.masks import make_identity

B, S, D, H = 2, 2048, 1024, 16
DEPTH = 64
NCORES = 8
HPC = 4               # heads per core
FS = HPC * DEPTH      # 256 features per core
QB = 512              # q block (matmul moving dim)
NQB = S // QB         # 4
NKT = S // 128        # 16 k tiles
IC = D // 128         # 8 input chunks
SCALE = 1.0 / np.sqrt(DEPTH)

F32 = mybir.dt.float32
BF16 = mybir.dt.float16
NPBF16 = np.float16

RUN_KWARGS = {}       # test.py may inject trace=True etc.
_CACHE = {}


def build():
    nc = bacc.Bacc("TRN2", target_bir_lowering=False)

    xq = nc.dram_tensor("xq", [IC, NQB // 2, 128, 2 * QB], BF16, kind="ExternalInput")
    xk = nc.dram_tensor("xk", [IC, NQB // 2, 128, 2 * QB], BF16, kind="ExternalInput")
    xv = nc.dram_tensor("xv", [IC, NQB // 2, 128, 2 * QB], BF16, kind="ExternalInput")
    wqT = nc.dram_tensor("wqT", [D, FS], BF16, kind="ExternalInput")
    wkT = nc.dram_tensor("wkT", [D, FS], BF16, kind="ExternalInput")
    wvT = nc.dram_tensor("wvT", [D, FS], BF16, kind="ExternalInput")
    fcT = nc.dram_tensor("fcT", [FS, D], BF16, kind="ExternalInput")
    qbv = nc.dram_tensor("qbv", [128, 2], F32, kind="ExternalInput")
    kbv = nc.dram_tensor("kbv", [128, 2], F32, kind="ExternalInput")
    vbv = nc.dram_tensor("vbv", [FS], F32, kind="ExternalInput")
    fco = nc.dram_tensor("fco", [NKT, 128, 2, QB], BF16, kind="ExternalOutput")
    atp = nc.dram_tensor("atp", [S, HPC, DEPTH], F32, kind="ExternalOutput")

    with tile.TileContext(nc) as tc:
        with tc.tile_pool(name="persist", bufs=1) as pp:
            wq_sb = pp.tile([128, IC, FS], BF16, name="wq_sb")
            wk_sb = pp.tile([128, IC, FS], BF16, name="wk_sb")
            wv_sb = pp.tile([128, IC, FS], BF16, name="wv_sb")
            fc_sb = pp.tile([128, 2, D], BF16, name="fc_sb")
            qbias = pp.tile([128, 2], F32, name="qbias")
            kbias = pp.tile([128, 2], F32, name="kbias")
            vbias_bc = pp.tile([128, FS], F32, name="vbias_bc")
            ident = pp.tile([128, 64], F32, name="ident")
            qh = [pp.tile([128, S], BF16, name=f"qh{t}") for t in range(2)]
            kh = [pp.tile([128, S], BF16, name=f"kh{t}") for t in range(2)]
            cc32 = [pp.tile([128, S], F32, name=f"cc32_{t}") for t in range(2)]
            cc16 = [pp.tile([128, S], BF16, name=f"cc16_{t}") for t in range(2)]
            vh_sb = pp.tile([128, NKT, HPC, DEPTH + 1], BF16, name="vh_sb")

            for i in range(IC):
                nc.sync.dma_start(out=wk_sb[:, i, :], in_=wkT[ts(i, 128), :])
            nc.sync.dma_start(out=kbias[:, :], in_=kbv[:, :])
            nc.gpsimd.dma_start(out=vbias_bc[:, :], in_=vbv[None, :].to_broadcast([128, FS]))
            make_identity(nc, ident[0:64, :])
            nc.vector.tensor_copy(ident[64:128, :], ident[0:64, :])
            nc.vector.memset(vh_sb[:, :, :, DEPTH:DEPTH + 1], 1.0)

            # ---- Phase A: projections ----
            with (
                tc.tile_pool(name="xin", bufs=3) as xp,
                tc.tile_pool(name="psA", bufs=2, space="PSUM") as psA,
                tc.tile_pool(name="psV", bufs=2, space="PSUM") as psV,
            ):
                for sbp in range(NQB // 2):
                    x_t = xp.tile([128, IC, 2 * QB], BF16, name="x_t", tag="xin")
                    for i in range(IC):
                        nc.sync.dma_start(out=x_t[:, i, :], in_=xk[i, sbp, :, :])
                    for half in range(2):
                        sb_i = 2 * sbp + half
                        for t in range(2):
                            ps = psA.tile([128, QB], F32, name="ps_t", tag="psA")
                            for i in range(IC):
                                nc.tensor.matmul(ps[:, :], wk_sb[:, i, ts(t, 128)],
                                                 x_t[:, i, ts(half, QB)],
                                                 start=(i == 0), stop=(i == IC - 1))
                            nc.vector.tensor_scalar_add(kh[t][:, ts(sb_i, QB)], ps[:, :], kbias[:, t:t + 1])
                for i in range(IC):
                    nc.sync.dma_start(out=wv_sb[:, i, :], in_=wvT[ts(i, 128), :])
                for sbp in range(NQB // 2):
                    x_t = xp.tile([128, IC, 2 * QB], BF16, name="x_t", tag="xin")
                    for i in range(IC):
                        nc.sync.dma_start(out=x_t[:, i, :], in_=xv[i, sbp, :, :])
                    for st in range(2 * QB // 128):
                        psv = psV.tile([128, FS], F32, name="psv_t", tag="psV")
                        for i in range(IC):
                            nc.tensor.matmul(psv[:, :], x_t[:, i, ts(st, 128)], wv_sb[:, i, :],
                                             start=(i == 0), stop=(i == IC - 1))
                        kt = sbp * (2 * QB // 128) + st
                        nc.vector.tensor_add(
                            vh_sb[:, kt, :, 0:DEPTH],
                            psv.rearrange("p (h d) -> p h d", h=HPC),
                            vbias_bc.rearrange("p (h d) -> p h d", h=HPC),
                        )
                for i in range(IC):
                    nc.sync.dma_start(out=wq_sb[:, i, :], in_=wqT[ts(i, 128), :])
                nc.sync.dma_start(out=qbias[:, :], in_=qbv[:, :])
                for i in range(2):
                    nc.sync.dma_start(out=fc_sb[:, i, :], in_=fcT[ts(i, 128), :])
                for sbp in range(NQB // 2):
                    x_t = xp.tile([128, IC, 2 * QB], BF16, name="x_t", tag="xin")
                    for i in range(IC):
                        nc.sync.dma_start(out=x_t[:, i, :], in_=xq[i, sbp, :, :])
                    for half in range(2):
                        sb_i = 2 * sbp + half
                        for t in range(2):
                            ps = psA.tile([128, QB], F32, name="ps_t", tag="psA")
                            for i in range(IC):
                                nc.tensor.matmul(ps[:, :], wq_sb[:, i, ts(t, 128)],
                                                 x_t[:, i, ts(half, QB)],
                                                 start=(i == 0), stop=(i == IC - 1))
                            nc.vector.tensor_scalar_add(qh[t][:, ts(sb_i, QB)], ps[:, :], qbias[:, t:t + 1])

            # ---- Phase B: attention, head pairs concurrent in the PE array ----
            with (
                tc.tile_pool(name="psS", bufs=2, space="PSUM") as psS,
                tc.tile_pool(name="psAV", bufs=4, space="PSUM") as psAV,
                tc.tile_pool(name="pP", bufs=4) as pP,
                tc.tile_pool(name="pR", bufs=4) as pR,
            ):
                for qb_i in range(NQB):
                    for hp in range(2):
                        h0, h1 = 2 * hp, 2 * hp + 1
                        av0 = psAV.tile([DEPTH + 1, QB], F32, name="av0_t", tag="av")
                        av1 = psAV.tile([DEPTH + 1, QB], F32, name="av1_t", tag="av")
                        for kt in range(NKT):
                            sp = psS.tile([128, 2 * QB], F32, name="sp_t", tag="s")
                            nc.tensor.matmul(sp[:, 0:QB], kh[hp][0:64, ts(kt, 128)],
                                             qh[hp][0:64, ts(qb_i, QB)],
                                             start=True, stop=True)
                            nc.tensor.matmul(sp[:, QB:2 * QB], kh[hp][64:128, ts(kt, 128)],
                                             qh[hp][64:128, ts(qb_i, QB)],
                                             start=True, stop=True)
                            p_t = pP.tile([128, 2 * QB], BF16, name="p_t", tag="p")
                            nc.scalar.activation(out=p_t[:, :], in_=sp[:, :],
                                                 func=mybir.ActivationFunctionType.Exp,
                                                 scale=float(SCALE))
                            nc.tensor.matmul(av0[:, :], vh_sb[:, kt, h0, :], p_t[:, 0:QB],
                                             start=(kt == 0), stop=(kt == NKT - 1))
                            nc.tensor.matmul(av1[:, :], vh_sb[:, kt, h1, :], p_t[:, QB:2 * QB],
                                             start=(kt == 0), stop=(kt == NKT - 1))
                        for h, av in ((h0, av0), (h1, av1)):
                            po = 64 * (h % 2)
                            d0 = pR.tile([1, QB], F32, name="d0_t", tag="d0")
                            nc.vector.tensor_copy(d0[0:1, :], av[DEPTH:DEPTH + 1, :])
                            rb = pR.tile([64, QB], F32, name="rb_t", tag="rb")
                            nc.gpsimd.partition_broadcast(rb[:, :], d0[0:1, :])
                            rr = pR.tile([64, QB], F32, name="rr_t", tag="rr")
                            nc.vector.reciprocal_approx_fast(out=rr[:, :], in_=rb[:, :])
                            nc.vector.tensor_mul(cc32[hp][po:po + 64, ts(qb_i, QB)],
                                                 av[0:DEPTH, :], rr[:, :])
                        nc.vector.tensor_copy(cc16[hp][:, ts(qb_i, QB)],
                                              cc32[hp][:, ts(qb_i, QB)])

            # ---- Phase C: fc partial, Phase D: attn_products out ----
            with (
                tc.tile_pool(name="psF", bufs=3, space="PSUM") as psF,
                tc.tile_pool(name="psT", bufs=3, space="PSUM") as psT,
                tc.tile_pool(name="pO", bufs=3) as pO,
                tc.tile_pool(name="pA", bufs=3) as pA,
            ):
                for st in range(NKT):
                    o2 = pO.tile([128, 2, QB], BF16, name="o2", tag="o")
                    for oh in range(2):
                        fps = psF.tile([128, QB], F32, name="fps", tag="f")
                        for icc in range(2):
                            nc.tensor.matmul(fps[:, :], cc16[icc][:, ts(st, 128)],
                                             fc_sb[:, icc, ts(oh, QB)],
                                             start=(icc == 0), stop=(icc == 1))
                        nc.scalar.copy(o2[:, oh, :], fps[:, :])
                    nc.sync.dma_start(out=fco[st, :, :, :], in_=o2[:, :, :])
                    at_t = pA.tile([128, HPC, DEPTH], F32, name="at_t", tag="at")
                    for h in range(HPC):
                        t, po = h // 2, 64 * (h % 2)
                        tp = psT.tile([128, 64], F32, name="tp", tag="t")
                        nc.tensor.matmul(tp[:, :], cc32[t][po:po + 64, ts(st, 128)],
                                         ident[po:po + 64, :], is_transpose=True)
                        nc.vector.tensor_copy(at_t[:, h, :], tp[:, :])
                    nc.sync.dma_start(out=atp[ts(st, 128), :, :], in_=at_t[:, :, :])

    nc.finalize()
    return nc


def _get_nc():
    if "nc" not in _CACHE:
        _CACHE["nc"] = build()
    return _CACHE["nc"]


def _numpy_reference(q, k, v, mask, wq_w, wq_b, wk_w, wk_b, wv_w, wv_b, fc_w, fc_b):
    """Exact fallback for the (never expected) nonzero-mask case."""
    def split_heads(x):
        b, s, _ = x.shape
        return x.reshape(b, s, H, DEPTH).transpose(0, 2, 1, 3)

    qh = split_heads(q @ wq_w.T + wq_b)
    kh = split_heads(k @ wk_w.T + wk_b)
    vh = split_heads(v @ wv_w.T + wv_b)
    scores = np.einsum("bhqd,bhkd->bhqk", qh, kh) / np.sqrt(DEPTH)
    scores = scores + mask * (-1e9)
    scores -= scores.max(axis=-1, keepdims=True)
    e = np.exp(scores)
    attn = e / e.sum(axis=-1, keepdims=True)
    ap = np.einsum("bhqk,bhkd->bhqd", attn, vh).transpose(0, 2, 1, 3)
    concat = ap.reshape(q.shape[0], q.shape[1], D)
    return (concat @ fc_w.T + fc_b).astype(np.float32), ap.astype(np.float32)


def kernel(q, k, v, mask, wq_w, wq_b, wk_w, wk_b, wv_w, wv_b, fc_w, fc_b):
    q = np.asarray(q, np.float32)
    k = np.asarray(k, np.float32)
    v = np.asarray(v, np.float32)
    mask = np.asarray(mask, np.float32)
    wq_w = np.asarray(wq_w, np.float32)
    wq_b = np.asarray(wq_b, np.float32)
    wk_w = np.asarray(wk_w, np.float32)
    wk_b = np.asarray(wk_b, np.float32)
    wv_w = np.asarray(wv_w, np.float32)
    wv_b = np.asarray(wv_b, np.float32)
    fc_w = np.asarray(fc_w, np.float32)
    fc_b = np.asarray(fc_b, np.float32)

    if mask.any():
        return _numpy_reference(q, k, v, mask, wq_w, wq_b, wk_w, wk_b,
                                wv_w, wv_b, fc_w, fc_b)

    def blk(x):
        # [S, D] -> feature-major paired blocks [IC, NQB//2, 128, 2*QB]
        xt = x.T.astype(NPBF16)
        return np.ascontiguousarray(
            xt.reshape(IC, 128, NQB // 2, 2 * QB).transpose(0, 2, 1, 3))

    qT = [blk(q[b]) for b in range(B)]
    kT = [blk(k[b]) for b in range(B)]
    vT = [blk(v[b]) for b in range(B)]
    per_g = []
    for g in range(NCORES // B):
        sl = slice(FS * g, FS * g + FS)
        per_g.append({
            "wqT": np.ascontiguousarray(wq_w[sl, :].T).astype(NPBF16),
            "wkT": np.ascontiguousarray(wk_w[sl, :].T).astype(NPBF16),
            "wvT": np.ascontiguousarray(wv_w[sl, :].T).astype(NPBF16),
            "fcT": np.ascontiguousarray(fc_w[:, sl].T).astype(NPBF16),
            "qbv": np.ascontiguousarray(wq_b[sl].reshape(2, 128).T),
            "kbv": np.ascontiguousarray(wk_b[sl].reshape(2, 128).T),
            "vbv": np.ascontiguousarray(wv_b[sl]),
        })
    in_maps = []
    for c in range(NCORES):
        b, g = c // 4, c % 4
        m = {"xq": qT[b], "xk": kT[b], "xv": vT[b]}
        m.update(per_g[g])
        in_maps.append(m)

    res = bass_utils.run_bass_kernel_spmd(_get_nc(), in_maps,
                                          core_ids=list(range(NCORES)),
                                          **RUN_KWARGS)
    _CACHE["last_result"] = res

    outputs = np.zeros((B, S, D), np.float32)
    attn = np.empty((B, S, H, DEPTH), np.float32)
    for c in range(NCORES):
        b, g = c // 4, c % 4
        outputs[b] += res.results[c]["fco"].reshape(S, D).astype(np.float32)
        attn[b, :, HPC * g:HPC * g + HPC, :] = res.results[c]["atp"]
    outputs += fc_b
    return outputs, attn
